# revision 25
# baseline (speedup 1.0000x reference)
"""Trainium2 Bass kernel for nn_MultiHeadAttention (B=2, S=2048, D=1024, H=16, causal).

Strategy (v4): shard by (batch x head-quarter). Cores 0-3 take batch 0,
cores 4-7 batch 1; each core computes 4 heads (2 head-pairs) end-to-end over
its batch's 2048 tokens.
  - QKV projections for its head slice, causal flash-style attention
    (exp without max-subtraction -- scores are ~N(0,1)), partial output
    projection against its w_o row-slice (256 rows, accumulated over the
    core's two head-pairs). Host sums each batch group's 4 bf16 partials
    in fp32 (the post-w_o all-reduce); zero on-device collectives.
  - fp8 DoubleRow (256-deep contraction, 2x MACs/cycle) for the bulk of the
    PE work: off-diagonal attention*V uses fp8 probs (EX2) x fp8 V (V2) over
    key-block PAIRS; the output projection for tokens >=512 uses fp8 CTX2
    (x8) x fp8 w_o (x16) with a 1/128 rescale at evacuation. Diagonal-band
    blocks and the first 512 tokens stay bf16 (few-key rows have no
    averaging to absorb fp8 noise).
  - PE p-state care: dummy-matmul warmup ramps the clock while the first
    slabs stream in; projection/output-projection work is split into
    ~0.4-0.9us quanta budget-paced uniformly across all 80 attention
    block-iterations (with dependency-aware forcing) so the PE never idles
    long enough to down-clock during the exp-paced phase.
  - Engine roles: PE matmuls; ACT = exp only; DVE = casts/normalize/mask;
    Pool(gpsimd) = k-load SWDGE, broadcasts, stores (cannot touch PSUM).

Self-contained: hardcodes shapes; no sibling imports.
"""

import sys

if "/opt/trn_rl_repo" not in sys.path:
    sys.path.insert(0, "/opt/trn_rl_repo")

import numpy as np

import concourse.bass as bass
import concourse.mybir as mybir
import concourse.tile as tile
from concourse import bacc
from concourse.bass_utils import run_bass_kernel_spmd

B, S, D, H = 2, 2048, 1024, 16
DK = D // H          # 64 head dim
N_CORES = 8
HPC = 4              # heads per core
DPC = DK * HPC       # 256 local feature columns per core
T = S                # 2048 tokens per core (its batch)
NTB = 16             # 128-token key blocks per pair-sequence
SCALE = 1.0 / np.sqrt(np.float32(DK))

f32 = mybir.dt.float32
bf16 = mybir.dt.bfloat16
fp8 = mybir.dt.float8e4

_CACHED = {}


def build_nc():
    nc = bacc.Bacc("TRN2", target_bir_lowering=False, debug=False, num_devices=N_CORES)

    # x tensors: row = 128*tcn + partition(d_low), cols = 8 feat-blocks * 512 tok
    qT = nc.dram_tensor("qT", [512, 4096], bf16, kind="ExternalInput")
    kT = nc.dram_tensor("kT", [512, 4096], bf16, kind="ExternalInput")
    vT = nc.dram_tensor("vT", [512, 4096], bf16, kind="ExternalInput")
    # weights partition-major: [128, pair(2) * featblock(8) * 128]
    wqT = nc.dram_tensor("wqT", [128, 2048], bf16, kind="ExternalInput")
    wkT = nc.dram_tensor("wkT", [128, 2048], bf16, kind="ExternalInput")
    wvT = nc.dram_tensor("wvT", [128, 2048], bf16, kind="ExternalInput")
    # w_o row-slice: bf16 [128, pair*1024] and fp8 x16 copy
    woT = nc.dram_tensor("woT", [128, 2048], bf16, kind="ExternalInput")
    wo8T = nc.dram_tensor("wo8T", [128, 2048], fp8, kind="ExternalInput")
    msk = nc.dram_tensor("msk", [128, 128], bf16, kind="ExternalInput")
    # output: row = 128*group+partition, cols = 4 tok-blocks * 1024 feat
    outp = nc.dram_tensor("outp", [512, 4096], bf16, kind="ExternalOutput")

    Exp = mybir.ActivationFunctionType.Exp
    CopyFn = mybir.ActivationFunctionType.Copy
    MUL = mybir.AluOpType.mult
    DR = mybir.MatmulPerfMode.DoubleRow

    with tile.TileContext(nc) as tc:
        with (
            tc.tile_pool(name="res", bufs=1) as res,          # resident SBUF
            tc.tile_pool(name="xq", bufs=3) as xq_pool,       # q token slabs (per tcn)
            tc.tile_pool(name="xk", bufs=3) as xk_pool,
            tc.tile_pool(name="xv", bufs=3) as xv_pool,
            tc.tile_pool(name="ex", bufs=4) as ex_pool,       # diag exp tiles (bf16)
            tc.tile_pool(name="ex2", bufs=4) as ex2_pool,     # off-diag exp pairs (fp8)
            tc.tile_pool(name="dv", bufs=2) as dv_pool,       # recip/bcast
            tc.tile_pool(name="nrm", bufs=4) as nrm_pool,     # deferred normalize
            tc.tile_pool(name="ob", bufs=2) as ob_pool,       # ph3 output staging
            tc.tile_pool(name="p1", bufs=2, space="PSUM") as p1,      # proj + o-proj: 2 banks
            tc.tile_pool(name="psc", bufs=2, space="PSUM") as psc,    # scores: 4 banks
            tc.tile_pool(name="pcx", bufs=2, space="PSUM") as pcx,    # ctx accumulators: 2 banks
        ):
            # ---------------- prelude ----------------
            # PE warmup: dummy matmuls ramp the tensor-engine p-state while
            # the first DMAs stream in (nothing reads the result).
            wm_sb = res.tile([128, 128], bf16, tag="wm")
            nc.vector.memset(wm_sb[:], 0.0)
            # exp bias tile: exp(s/8 - 3.5) keeps probs inside fp8 range;
            # the constant cancels between numerator and denominator
            bias_sb = res.tile([128, 1], f32, tag="bias")
            nc.vector.memset(bias_sb[:], -3.5)
            wm_ps = p1.tile([128, 128], f32, tag="p1", name="wm")
            NWARM = 170
            for i in range(NWARM):
                nc.tensor.matmul(wm_ps[:], wm_sb[:], wm_sb[:],
                                 start=(i == 0), stop=(i == NWARM - 1))

            # weights + mask lead their queues ahead of the token slabs
            wq_sb = res.tile([128, 2, 8, 128], bf16, tag="wq")
            nc.sync.dma_start(out=wq_sb[:], in_=wqT[:])
            wk_sb = res.tile([128, 2, 8, 128], bf16, tag="wk")
            nc.gpsimd.dma_start(out=wk_sb[:], in_=wkT[:])
            mk_sb = res.tile([128, 128], bf16, tag="mk")
            nc.gpsimd.dma_start(out=mk_sb[:], in_=msk[:])
            wv_sb = res.tile([128, 2, 8, 128], bf16, tag="wv")
            nc.sync.dma_start(out=wv_sb[:], in_=wvT[:])

            def ph1_loads(tcn):
                """Slab DMAs for one 512-token chunk; q/k/v on 3 queues."""
                qt = xq_pool.tile([128, 8, 512], bf16, tag="xq", name="qt")
                kt = xk_pool.tile([128, 8, 512], bf16, tag="xk", name="kt")
                vt = xv_pool.tile([128, 8, 512], bf16, tag="xv", name="vt")
                rows = slice(128 * tcn, 128 * (tcn + 1))
                nc.sync.dma_start(out=qt[:], in_=qT[rows, :])
                nc.gpsimd.dma_start(out=kt[:], in_=kT[rows, :])
                nc.sync.dma_start(out=vt[:], in_=vT[rows, :])
                return qt, kt, vt

            tiles = [ph1_loads(0), ph1_loads(1)]

            wo_sb = res.tile([128, 2, 1024], bf16, tag="wo")
            nc.gpsimd.dma_start(out=wo_sb[:], in_=woT[:])
            wo8_sb = res.tile([128, 2, 1024], fp8, tag="wo8")
            nc.sync.dma_start(out=wo8_sb[:], in_=wo8T[:])

            tiles += [ph1_loads(2), ph1_loads(3)]

            # resident activations
            QHT = res.tile([128, 2 * T], bf16, tag="QHT")   # [d_local, pair*2048]
            KHT = res.tile([128, 2 * T], bf16, tag="KHT")
            # bf16 V for diagonal blocks: per (pair, blk): 2 heads x (64 + ones)
            V_sb = res.tile([128, 2 * NTB * 130], bf16, tag="V")
            # fp8 V for off-diag DoubleRow: per (pair, blkpair, head):
            # [r(2) x 128] where col 64 = ones, 65..127 = 0
            V2 = res.tile([128, 2, 8, 2, 2, 128], fp8, tag="V2")  # pair, bp, head, r, col
            CTX = res.tile([128, 2, 512], bf16, tag="CTX")  # first 512 tokens only
            CTX2 = res.tile([128, 2, T], fp8, tag="CTX2")  # [d, pair, tok] x8

            nc.vector.memset(
                V_sb[:].rearrange("p (n x) -> p n x", x=65)[:, :, 64:65], 1.0
            )
            nc.vector.memset(V2[:, :, :, :, :, 64:], 0.0)
            nc.vector.memset(V2[:, :, :, :, :, 64:65], 1.0)

            # ---- filler machinery
            filler = []  # list of [kind, pair, tcn, cost_us, fn]
            carry = {"us": 0.0}

            def emit_budget(us):
                carry["us"] += us
                while filler and filler[0][3] <= carry["us"]:
                    item = filler.pop(0)
                    carry["us"] -= item[3]
                    item[4]()

            def force_sel(kind, pair, upto_tcn):
                i = 0
                while i < len(filler):
                    k2, p2, t2, _, fn = filler[i]
                    if k2 == kind and p2 == pair and t2 <= upto_tcn:
                        filler.pop(i)
                        fn()
                    else:
                        i += 1

            def make_quanta(tcn, pair):
                """Six ~0.85us projection quanta for (tcn, pair)."""
                qt, kt, vt = tiles[tcn]
                cols = slice(2048 * pair + 512 * tcn, 2048 * pair + 512 * (tcn + 1))
                out = []

                def xy_quant(w_sb, dst, lo, hi, xt):
                    ps = p1.tile([128, 256], f32, tag="p1", name="ps_p")
                    for kb in range(8):
                        nc.tensor.matmul(ps[:], w_sb[:, pair, kb, :],
                                         xt[:, kb, lo:hi], start=kb == 0, stop=kb == 7)
                    nc.vector.tensor_copy(dst[:, cols.start + lo:cols.start + hi], ps[:])

                for lo in (0, 256):
                    out.append(["q", pair, tcn, 0.85,
                                lambda lo=lo: xy_quant(wq_sb, QHT, lo, lo + 256, qt)])
                for lo in (0, 256):
                    out.append(["k", pair, tcn, 0.85,
                                lambda lo=lo: xy_quant(wk_sb, KHT, lo, lo + 256, kt)])

                def v_quant(i0):
                    ps_v = p1.tile([128, 256], f32, tag="p1", name="ps_v")
                    for i in (i0, i0 + 1):
                        for kb in range(8):
                            nc.tensor.matmul(
                                ps_v[:, 128 * (i - i0):128 * (i - i0 + 1)],
                                vt[:, kb, 128 * i:128 * (i + 1)],
                                wv_sb[:, pair, kb, :],
                                start=(kb == 0), stop=(kb == 7),
                            )
                    for i in (i0, i0 + 1):
                        j = 4 * tcn + i
                        g = 16 * pair + j
                        ps3 = ps_v[:, 128 * (i - i0):128 * (i - i0 + 1)].rearrange(
                            "p (h x) -> p h x", x=64)
                        nc.vector.tensor_copy(
                            V_sb[:, 130 * g:130 * (g + 1)].rearrange(
                                "p (h x) -> p h x", x=65)[:, :, 0:64],
                            ps3,
                        )
                        # fp8 copy for DoubleRow: [pair, bp, head, r, 0:64]
                        nc.vector.tensor_copy(
                            V2[:, pair, j // 2, :, j % 2, 0:64], ps3)

                for i0 in (0, 2):
                    out.append(["v", pair, tcn, 0.85, lambda i0=i0: v_quant(i0)])
                return out

            TOTAL_BLOCKS = 2 * (4 + 8 + 12 + 16)   # 80
            state = {"blocks_left": TOTAL_BLOCKS}

            def ph2_chunk(pair, c, nxt=None):
                """Causal attention for both heads of this pair, query chunk c (512 q)."""
                force_sel("q", pair, c)
                qcols = slice(2048 * pair + 512 * c, 2048 * pair + 512 * (c + 1))
                ps_ctx = {}
                for h in range(2):
                    ps_ctx[h] = pcx.tile([128, 512], f32, tag="ctx", name="ps_ctx")
                nblk = 4 * c + 4
                ndiag = 4 * c  # blocks 0..4c-1 are off-diagonal (full band)

                def emit_ctx(p):
                    if p[0] == "dr":
                        _, bp, ex2 = p
                        force_sel("v", pair, (2 * bp + 1) // 4)
                        for h in range(2):
                            nc.tensor.matmul(
                                ps_ctx[h][:, :],
                                V2[:, pair, bp, h],
                                ex2[:, :, h],
                                start=(bp == 0), stop=False,
                                perf_mode=DR,
                            )
                    else:
                        _, j, band, ex = p
                        force_sel("v", pair, j // 4)
                        g = 16 * pair + j
                        for h in range(2):
                            nc.tensor.matmul(
                                ps_ctx[h][0:65, band],
                                V_sb[:, 130 * g + 65 * h:130 * g + 65 * (h + 1)],
                                ex[:, 512 * h + band.start:512 * h + band.stop],
                                start=(c == 0 and j == 0), stop=(j == nblk - 1),
                            )

                ex2 = None
                pendq = []
                for j in range(nblk):
                    force_sel("k", pair, j // 4)
                    kcols = slice(2048 * pair + 128 * j, 2048 * pair + 128 * (j + 1))
                    d = j - 4 * c
                    band = slice(128 * d, 512) if d > 0 else slice(0, 512)
                    qb = slice(qcols.start + band.start, qcols.stop)
                    sc = psc.tile([128, 1024], f32, tag="sc", name="sc")
                    for h in range(2):
                        rows = slice(64 * h, 64 * (h + 1))
                        nc.tensor.matmul(
                            sc[:, 512 * h + band.start:512 * h + band.stop],
                            KHT[rows, kcols], QHT[rows, qb], start=True, stop=True)
                    if j < ndiag:
                        # off-diagonal: exp into fp8 pair tile [r, h, q]
                        if j % 2 == 0:
                            ex2 = ex2_pool.tile([128, 2, 2, 512], fp8, tag="ex2", name="ex2")
                        nc.scalar.activation(
                            ex2[:, j % 2].rearrange("p h q -> p (h q)"), sc[:],
                            Exp, bias=bias_sb[:], scale=float(SCALE))
                    else:
                        ex = ex_pool.tile([128, 1024], bf16, tag="ex", name="ex")
                        if band.start == 0:
                            nc.scalar.activation(ex[:], sc[:], Exp, bias=bias_sb[:], scale=float(SCALE))
                        else:
                            sc3 = sc[:].rearrange("p (h q) -> p h q", h=2)[:, :, band.start:band.stop]
                            ex3 = ex[:].rearrange("p (h q) -> p h q", h=2)[:, :, band.start:band.stop]
                            nc.scalar.activation(ex3, sc3, Exp, bias=bias_sb[:], scale=float(SCALE))
                        if d >= 0:
                            mband = slice(128 * d, 128 * (d + 1))
                            exm = ex[:].rearrange("p (h q) -> p h q", h=2)[:, :, mband]
                            mk3 = mk_sb[:].rearrange("p (h q) -> p h q", h=1).broadcast_to((128, 2, 128))
                            nc.vector.tensor_tensor(exm, exm, mk3, MUL)
                    ready = None
                    if j < ndiag:
                        if j % 2 == 1:
                            ready = ("dr", j // 2, ex2)
                    else:
                        ready = ("diag", j, band, ex)
                    if ready is not None:
                        pendq.append(ready)
                        if len(pendq) > 1:
                            bl = state["blocks_left"]
                            if bl > 1:
                                f = 1.6 if state["blocks_left"] > TOTAL_BLOCKS - 24 else 1.0
                                emit_budget(f * sum(it[3] for it in filler) / bl)
                            emit_ctx(pendq.pop(0))
                    state["blocks_left"] -= 1
                    if j == nblk - 1 and nxt is not None:
                        # next chunk's q/k projections run in this chunk's
                        # boundary lull (pend flush + normalize latency)
                        force_sel("q", nxt[0], nxt[1])
                        force_sel("k", nxt[0], nxt[1])
                for p in pendq:
                    emit_ctx(p)
                # normalize: evacuate the ctx psum to SBUF immediately (frees the
                # pcx bank for the next chunk), then run the recip/broadcast/mult
                # chain out of SBUF off the PE's critical path.
                # c=0 -> bf16 CTX (unscaled); c>=1 -> fp8 CTX2 (x8)
                craw = {}
                bcs = {}
                for h in range(2):
                    craw[h] = nrm_pool.tile([65, 512], f32, tag="craw", name="craw")
                    nc.vector.tensor_copy(craw[h][:], ps_ctx[h][0:65, :])
                for h in range(2):
                    den = dv_pool.tile([1, 512], f32, tag="den")
                    if c == 0:
                        nc.vector.tensor_copy(den[:], craw[h][64:65, :])
                    else:
                        nc.vector.tensor_scalar_mul(den[:], craw[h][64:65, :], 0.125)
                    rec = dv_pool.tile([1, 512], f32, tag="rec")
                    nc.vector.reciprocal_approx_fast(out=rec[:], in_=den[:])
                    bcs[h] = nrm_pool.tile([64, 512], f32, tag="bc", name="bc")
                    nc.gpsimd.partition_broadcast(bcs[h][:], rec[:])
                # the multiplies are deferred into the filler stream so the DVE
                # queue never blocks waiting on the pool broadcast at a chunk
                # boundary (the next chunk's mask-mults would convoy behind it)
                def norm_mult(pair=pair, c=c, craw=craw, bcs=bcs):
                    for h in range(2):
                        rows = slice(64 * h, 64 * (h + 1))
                        if c == 0:
                            nc.vector.tensor_tensor(CTX[rows, pair, :], craw[h][0:64, :], bcs[h][:], MUL)
                        else:
                            nc.vector.tensor_tensor(
                                CTX2[rows, pair, 512 * c:512 * (c + 1)],
                                craw[h][0:64, :], bcs[h][:], MUL)
                filler.append(["n", pair, c, 0.1, norm_mult])
                # both pairs' chunk c done -> output projection quanta
                if chunks_done[c] == 2:
                    obholder = {}
                    last = (c == CHUNK_ORDER[-1][1])
                    for tb in range(4 * c, 4 * (c + 1)):
                        for e in range(2):
                            def ph3_quant(tb=tb, e=e, obholder=obholder, last=last):
                                force_sel("n", 0, 3)
                                force_sel("n", 1, 3)
                                grp, idx = divmod(tb, 4)
                                if idx == 0 and e == 0:
                                    obholder["ob"] = ob_pool.tile([128, 4, 1024], bf16, tag="ob", name="ob")
                                ob = obholder["ob"]
                                po = p1.tile([128, 512], f32, tag="p1", name="po")
                                if tb < 4:
                                    for pr in range(2):
                                        nc.tensor.matmul(
                                            po[:],
                                            CTX[:, pr, 128 * tb:128 * (tb + 1)],
                                            wo_sb[:, pr, 512 * e:512 * (e + 1)],
                                            start=(pr == 0), stop=(pr == 1),
                                        )
                                    if last:
                                        nc.scalar.activation(ob[:, idx, 512 * e:512 * (e + 1)], po[:], CopyFn)
                                    else:
                                        nc.vector.tensor_copy(ob[:, idx, 512 * e:512 * (e + 1)], po[:])
                                else:
                                    nc.tensor.matmul(
                                        po[:],
                                        CTX2[:, :, 128 * tb:128 * (tb + 1)],
                                        wo8_sb[:, :, 512 * e:512 * (e + 1)],
                                        start=True, stop=True,
                                        perf_mode=DR,
                                    )
                                    if last:
                                        nc.scalar.activation(
                                            ob[:, idx, 512 * e:512 * (e + 1)], po[:], CopyFn,
                                            scale=1.0 / 128.0)
                                    else:
                                        nc.vector.tensor_scalar_mul(
                                            ob[:, idx, 512 * e:512 * (e + 1)], po[:], 1.0 / 128.0)
                                if e == 1:
                                    nc.gpsimd.dma_start(
                                        out=outp[128 * grp:128 * (grp + 1), 1024 * idx:1024 * (idx + 1)],
                                        in_=ob[:, idx])
                            filler.append(["o", -1, 99, 0.45, ph3_quant])

            # ---- schedule
            CHUNK_ORDER = [(0, 0), (1, 0), (0, 1), (1, 1), (0, 3), (1, 3), (0, 2), (1, 2)]
            chunks_done = {c: 0 for c in range(4)}
            for tcn in range(4):
                for pair in range(2):
                    quanta = make_quanta(tcn, pair)
                    if tcn == 0 and pair == 0:
                        for it in quanta:
                            if it[0] in ("q", "k"):
                                it[4]()
                            else:
                                filler.append(it)
                    else:
                        filler.extend(quanta)

            for i, (pair, c) in enumerate(CHUNK_ORDER):
                chunks_done[c] += 1
                nxt = CHUNK_ORDER[i + 1] if i + 1 < len(CHUNK_ORDER) else None
                ph2_chunk(pair, c, nxt)
            while filler:
                filler.pop(0)[4]()

    nc.compile()
    return nc


def _host_inputs(q, k, v, mask, w_q, w_k, w_v, w_o):
    import ml_dtypes

    nbf = ml_dtypes.bfloat16
    nf8 = getattr(ml_dtypes, "float8_e4m3fn", None) or ml_dtypes.float8_e4m3

    def arrange_x(x):
        # [T, D] tokens-major -> [tcn, partition(d_low), featblock(a)*token]
        x2 = np.asarray(x, dtype=np.float32).reshape(T, D).T.astype(nbf)  # [D, T]
        x4 = x2.reshape(8, 128, 4, 512)             # (a, p, tcn, t)
        return np.ascontiguousarray(x4.transpose(2, 1, 0, 3).reshape(512, 4096))

    def arrange_w(wT):
        # wT [D, DPC] -> [128, pair*8*128] partition-major image of the SBUF tile
        w4 = wT.astype(nbf).reshape(8, 128, 2, 128)  # (a, p, pair, d)
        return np.ascontiguousarray(w4.transpose(1, 2, 0, 3).reshape(128, 2048))

    w_q = np.asarray(w_q, dtype=np.float32)
    w_k = np.asarray(w_k, dtype=np.float32)
    w_v = np.asarray(w_v, dtype=np.float32)
    w_o = np.asarray(w_o, dtype=np.float32)
    mask2d = np.asarray(mask).reshape(S, S)

    # single 128x128 tril mask for the mixed band of every diagonal block:
    # valid(r, u) = mask2d[u, r] on the leading 128x128 (= u >= r for causal)
    mk = np.ascontiguousarray(mask2d[0:128, 0:128].T.astype(nbf))

    xarr = {}
    for b in range(B):
        xarr[b] = (
            arrange_x(np.asarray(q)[b]),
            arrange_x(np.asarray(k)[b]),
            arrange_x(np.asarray(v)[b]),
        )

    in_maps = []
    for m in range(N_CORES):
        b, g = divmod(m, 4)
        sl = slice(DPC * g, DPC * (g + 1))
        q4, k4, v4 = xarr[b]
        # w_o[:, sl].T is [256, 1024]; split into two 128-row pair slabs
        wo2 = w_o[:, sl].T.reshape(2, 128, 1024).transpose(1, 0, 2)
        in_maps.append({
            "qT": q4,
            "kT": k4,
            "vT": v4,
            "wqT": arrange_w(w_q[sl, :].T),
            "wkT": arrange_w(w_k[sl, :].T),
            "wvT": arrange_w(w_v[sl, :].T),
            "woT": np.ascontiguousarray(wo2.astype(nbf).reshape(128, 2048)),
            "wo8T": np.ascontiguousarray((wo2 * 16.0).astype(nf8).reshape(128, 2048)),
            "msk": mk,
        })
    return in_maps


def kernel(q, k, v, mask, w_q, w_k, w_v, w_o, _trace=False, _results=None):
    in_maps = _host_inputs(q, k, v, mask, w_q, w_k, w_v, w_o)
    if "nc" not in _CACHED:
        _CACHED["nc"] = build_nc()
    nc = _CACHED["nc"]
    res = run_bass_kernel_spmd(
        nc, in_maps, core_ids=list(range(N_CORES)), trace=_trace
    )
    if _results is not None:
        _results.append(res)
    out = np.zeros((B, 512, 4096), dtype=np.float32)
    for m in range(N_CORES):
        out[m // 4] += np.asarray(res.results[m]["outp"], dtype=np.float32)
    # [group, partition, tokblock*feat] -> [B, S, D]
    out = out.reshape(B, 4, 128, 4, 1024).transpose(0, 1, 3, 2, 4).reshape(B, S, D)
    return out


# revision 26
# speedup vs baseline: 1.0374x; 1.0374x over previous
"""Trainium2 Bass kernel for nn_MultiHeadAttention (B=2, S=2048, D=1024, H=16, causal).

Strategy (v4): shard by (batch x head-quarter). Cores 0-3 take batch 0,
cores 4-7 batch 1; each core computes 4 heads (2 head-pairs) end-to-end over
its batch's 2048 tokens.
  - QKV projections for its head slice, causal flash-style attention
    (exp without max-subtraction -- scores are ~N(0,1)), partial output
    projection against its w_o row-slice (256 rows, accumulated over the
    core's two head-pairs). Host sums each batch group's 4 bf16 partials
    in fp32 (the post-w_o all-reduce); zero on-device collectives.
  - fp8 DoubleRow (256-deep contraction, 2x MACs/cycle) for the bulk of the
    PE work: off-diagonal attention*V uses fp8 probs (EX2) x fp8 V (V2) over
    key-block PAIRS; the output projection for tokens >=512 uses fp8 CTX2
    (x8) x fp8 w_o (x16) with a 1/128 rescale at evacuation. Diagonal-band
    blocks and the first 512 tokens stay bf16 (few-key rows have no
    averaging to absorb fp8 noise).
  - PE p-state care: dummy-matmul warmup ramps the clock while the first
    slabs stream in; projection/output-projection work is split into
    ~0.4-0.9us quanta budget-paced uniformly across all 80 attention
    block-iterations (with dependency-aware forcing) so the PE never idles
    long enough to down-clock during the exp-paced phase.
  - Engine roles: PE matmuls; ACT = exp only; DVE = casts/normalize/mask;
    Pool(gpsimd) = k-load SWDGE, broadcasts, stores (cannot touch PSUM).

Self-contained: hardcodes shapes; no sibling imports.
"""

import sys

if "/opt/trn_rl_repo" not in sys.path:
    sys.path.insert(0, "/opt/trn_rl_repo")

import numpy as np

import concourse.bass as bass
import concourse.mybir as mybir
import concourse.tile as tile
from concourse import bacc
from concourse.bass_utils import run_bass_kernel_spmd

B, S, D, H = 2, 2048, 1024, 16
DK = D // H          # 64 head dim
N_CORES = 8
HPC = 4              # heads per core
DPC = DK * HPC       # 256 local feature columns per core
T = S                # 2048 tokens per core (its batch)
NTB = 16             # 128-token key blocks per pair-sequence
SCALE = 1.0 / np.sqrt(np.float32(DK))

f32 = mybir.dt.float32
bf16 = mybir.dt.bfloat16
fp8 = mybir.dt.float8e4

_CACHED = {}


def build_nc():
    nc = bacc.Bacc("TRN2", target_bir_lowering=False, debug=False, num_devices=N_CORES)

    # x tensors: row = 128*tcn + partition(d_low), cols = 8 feat-blocks * 512 tok
    qT = nc.dram_tensor("qT", [512, 4096], bf16, kind="ExternalInput")
    kT = nc.dram_tensor("kT", [512, 4096], bf16, kind="ExternalInput")
    vT = nc.dram_tensor("vT", [512, 4096], bf16, kind="ExternalInput")
    # weights partition-major: [128, pair(2) * featblock(8) * 128]
    wqT = nc.dram_tensor("wqT", [128, 2048], bf16, kind="ExternalInput")
    wkT = nc.dram_tensor("wkT", [128, 2048], bf16, kind="ExternalInput")
    wvT = nc.dram_tensor("wvT", [128, 2048], bf16, kind="ExternalInput")
    # w_o row-slice: bf16 [128, pair*1024] and fp8 x16 copy
    woT = nc.dram_tensor("woT", [128, 2048], bf16, kind="ExternalInput")
    wo8T = nc.dram_tensor("wo8T", [128, 2048], fp8, kind="ExternalInput")
    msk = nc.dram_tensor("msk", [128, 128], bf16, kind="ExternalInput")
    # output: row = 128*group+partition, cols = 4 tok-blocks * 1024 feat
    outp = nc.dram_tensor("outp", [512, 4096], bf16, kind="ExternalOutput")

    Exp = mybir.ActivationFunctionType.Exp
    CopyFn = mybir.ActivationFunctionType.Copy
    MUL = mybir.AluOpType.mult
    DR = mybir.MatmulPerfMode.DoubleRow

    with tile.TileContext(nc) as tc:
        with (
            tc.tile_pool(name="res", bufs=1) as res,          # resident SBUF
            tc.tile_pool(name="xq", bufs=3) as xq_pool,       # q token slabs (per tcn)
            tc.tile_pool(name="xk", bufs=3) as xk_pool,
            tc.tile_pool(name="xv", bufs=3) as xv_pool,
            tc.tile_pool(name="ex", bufs=4) as ex_pool,       # diag exp tiles (bf16)
            tc.tile_pool(name="ex2", bufs=4) as ex2_pool,     # off-diag exp pairs (fp8)
            tc.tile_pool(name="dv", bufs=2) as dv_pool,       # recip/bcast
            tc.tile_pool(name="nrm", bufs=4) as nrm_pool,     # deferred normalize
            tc.tile_pool(name="ob", bufs=2) as ob_pool,       # ph3 output staging
            tc.tile_pool(name="p1", bufs=2, space="PSUM") as p1,      # proj + o-proj: 2 banks
            tc.tile_pool(name="psc", bufs=2, space="PSUM") as psc,    # scores: 4 banks
            tc.tile_pool(name="pcx", bufs=2, space="PSUM") as pcx,    # ctx accumulators: 2 banks
        ):
            # ---------------- prelude ----------------
            # PE warmup: dummy matmuls ramp the tensor-engine p-state while
            # the first DMAs stream in (nothing reads the result).
            wm_sb = res.tile([128, 128], bf16, tag="wm")
            nc.vector.memset(wm_sb[:], 0.0)
            # exp bias tile: exp(s/8 - 3.5) keeps probs inside fp8 range;
            # the constant cancels between numerator and denominator
            bias_sb = res.tile([128, 1], f32, tag="bias")
            nc.vector.memset(bias_sb[:], -3.5)
            wm_ps = p1.tile([128, 128], f32, tag="p1", name="wm")
            NWARM = 170
            for i in range(NWARM):
                nc.tensor.matmul(wm_ps[:], wm_sb[:], wm_sb[:],
                                 start=(i == 0), stop=(i == NWARM - 1))

            # weights + mask lead their queues ahead of the token slabs
            wq_sb = res.tile([128, 2, 8, 128], bf16, tag="wq")
            nc.sync.dma_start(out=wq_sb[:], in_=wqT[:])
            wk_sb = res.tile([128, 2, 8, 128], bf16, tag="wk")
            nc.gpsimd.dma_start(out=wk_sb[:], in_=wkT[:])
            mk_sb = res.tile([128, 128], bf16, tag="mk")
            nc.gpsimd.dma_start(out=mk_sb[:], in_=msk[:])
            wv_sb = res.tile([128, 2, 8, 128], bf16, tag="wv")
            nc.sync.dma_start(out=wv_sb[:], in_=wvT[:])

            def ph1_loads(tcn):
                """Slab DMAs for one 512-token chunk; q/k/v on 3 queues."""
                qt = xq_pool.tile([128, 8, 512], bf16, tag="xq", name="qt")
                kt = xk_pool.tile([128, 8, 512], bf16, tag="xk", name="kt")
                vt = xv_pool.tile([128, 8, 512], bf16, tag="xv", name="vt")
                rows = slice(128 * tcn, 128 * (tcn + 1))
                nc.sync.dma_start(out=qt[:], in_=qT[rows, :])
                nc.gpsimd.dma_start(out=kt[:], in_=kT[rows, :])
                nc.sync.dma_start(out=vt[:], in_=vT[rows, :])
                return qt, kt, vt

            tiles = [ph1_loads(0), ph1_loads(1)]

            wo_sb = res.tile([128, 2, 1024], bf16, tag="wo")
            nc.gpsimd.dma_start(out=wo_sb[:], in_=woT[:])
            wo8_sb = res.tile([128, 2, 1024], fp8, tag="wo8")
            nc.sync.dma_start(out=wo8_sb[:], in_=wo8T[:])

            tiles += [ph1_loads(2), ph1_loads(3)]

            # resident activations
            QHT = res.tile([128, 2 * T], bf16, tag="QHT")   # [d_local, pair*2048]
            KHT = res.tile([128, 2 * T], bf16, tag="KHT")
            # bf16 V for diagonal blocks: per (pair, blk): 2 heads x (64 + ones)
            V_sb = res.tile([128, 2 * NTB * 130], bf16, tag="V")
            # fp8 V for off-diag DoubleRow: per (pair, blkpair, head):
            # [r(2) x 128] where col 64 = ones, 65..127 = 0
            V2 = res.tile([128, 2, 8, 2, 2, 128], fp8, tag="V2")  # pair, bp, head, r, col
            CTX = res.tile([128, 2, 512], bf16, tag="CTX")  # first 512 tokens only
            CTX2 = res.tile([128, 2, T], fp8, tag="CTX2")  # [d, pair, tok] x8

            nc.vector.memset(
                V_sb[:].rearrange("p (n x) -> p n x", x=65)[:, :, 64:65], 1.0
            )
            nc.vector.memset(V2[:, :, :, :, :, 64:], 0.0)
            nc.vector.memset(V2[:, :, :, :, :, 64:65], 1.0)

            # ---- filler machinery
            filler = []  # list of [kind, pair, tcn, cost_us, fn]
            carry = {"us": 0.0}

            def emit_budget(us):
                carry["us"] += us
                while filler and filler[0][3] <= carry["us"]:
                    item = filler.pop(0)
                    carry["us"] -= item[3]
                    item[4]()

            def force_sel(kind, pair, upto_tcn):
                i = 0
                while i < len(filler):
                    k2, p2, t2, _, fn = filler[i]
                    if k2 == kind and p2 == pair and t2 <= upto_tcn:
                        filler.pop(i)
                        fn()
                    else:
                        i += 1

            def make_quanta(tcn, pair):
                """Six ~0.85us projection quanta for (tcn, pair)."""
                qt, kt, vt = tiles[tcn]
                cols = slice(2048 * pair + 512 * tcn, 2048 * pair + 512 * (tcn + 1))
                out = []

                def xy_quant(w_sb, dst, lo, hi, xt):
                    ps = p1.tile([128, 256], f32, tag="p1", name="ps_p")
                    for kb in range(8):
                        nc.tensor.matmul(ps[:], w_sb[:, pair, kb, :],
                                         xt[:, kb, lo:hi], start=kb == 0, stop=kb == 7)
                    nc.vector.tensor_copy(dst[:, cols.start + lo:cols.start + hi], ps[:])

                for lo in (0, 256):
                    out.append(["q", pair, tcn, 0.85,
                                lambda lo=lo: xy_quant(wq_sb, QHT, lo, lo + 256, qt)])
                for lo in (0, 256):
                    out.append(["k", pair, tcn, 0.85,
                                lambda lo=lo: xy_quant(wk_sb, KHT, lo, lo + 256, kt)])

                def v_quant(i0):
                    ps_v = p1.tile([128, 256], f32, tag="p1", name="ps_v")
                    for i in (i0, i0 + 1):
                        for kb in range(8):
                            nc.tensor.matmul(
                                ps_v[:, 128 * (i - i0):128 * (i - i0 + 1)],
                                vt[:, kb, 128 * i:128 * (i + 1)],
                                wv_sb[:, pair, kb, :],
                                start=(kb == 0), stop=(kb == 7),
                            )
                    for i in (i0, i0 + 1):
                        j = 4 * tcn + i
                        g = 16 * pair + j
                        ps3 = ps_v[:, 128 * (i - i0):128 * (i - i0 + 1)].rearrange(
                            "p (h x) -> p h x", x=64)
                        nc.vector.tensor_copy(
                            V_sb[:, 130 * g:130 * (g + 1)].rearrange(
                                "p (h x) -> p h x", x=65)[:, :, 0:64],
                            ps3,
                        )
                        # fp8 copy for DoubleRow: [pair, bp, head, r, 0:64]
                        nc.vector.tensor_copy(
                            V2[:, pair, j // 2, :, j % 2, 0:64], ps3)

                for i0 in (0, 2):
                    out.append(["v", pair, tcn, 0.85, lambda i0=i0: v_quant(i0)])
                return out

            TOTAL_BLOCKS = 2 * (4 + 8 + 12 + 16)   # 80
            state = {"blocks_left": TOTAL_BLOCKS}

            def ph2_chunk(pair, c, nxt=None):
                """Causal attention for both heads of this pair, query chunk c (512 q)."""
                force_sel("q", pair, c)
                qcols = slice(2048 * pair + 512 * c, 2048 * pair + 512 * (c + 1))
                ps_ctx = {}
                for h in range(2):
                    ps_ctx[h] = pcx.tile([128, 512], f32, tag="ctx", name="ps_ctx")
                nblk = 4 * c + 4
                ndiag = 4 * c  # blocks 0..4c-1 are off-diagonal (full band)

                def emit_ctx(p):
                    if p[0] == "dr":
                        _, bp, ex2 = p
                        force_sel("v", pair, (2 * bp + 1) // 4)
                        for h in range(2):
                            nc.tensor.matmul(
                                ps_ctx[h][:, :],
                                V2[:, pair, bp, h],
                                ex2[:, :, h],
                                start=(bp == 0), stop=False,
                                perf_mode=DR,
                            )
                    else:
                        _, j, band, ex = p
                        force_sel("v", pair, j // 4)
                        g = 16 * pair + j
                        for h in range(2):
                            nc.tensor.matmul(
                                ps_ctx[h][0:65, band],
                                V_sb[:, 130 * g + 65 * h:130 * g + 65 * (h + 1)],
                                ex[:, 512 * h + band.start:512 * h + band.stop],
                                start=(c == 0 and j == 0), stop=(j == nblk - 1),
                            )

                ex2 = None
                pendq = []
                for j in range(nblk):
                    force_sel("k", pair, j // 4)
                    kcols = slice(2048 * pair + 128 * j, 2048 * pair + 128 * (j + 1))
                    d = j - 4 * c
                    band = slice(128 * d, 512) if d > 0 else slice(0, 512)
                    qb = slice(qcols.start + band.start, qcols.stop)
                    sc = psc.tile([128, 1024], f32, tag="sc", name="sc")
                    for h in range(2):
                        rows = slice(64 * h, 64 * (h + 1))
                        nc.tensor.matmul(
                            sc[:, 512 * h + band.start:512 * h + band.stop],
                            KHT[rows, kcols], QHT[rows, qb], start=True, stop=True)
                    if j < ndiag:
                        # off-diagonal: exp into fp8 pair tile [r, h, q]
                        if j % 2 == 0:
                            ex2 = ex2_pool.tile([128, 2, 2, 512], fp8, tag="ex2", name="ex2")
                        nc.scalar.activation(
                            ex2[:, j % 2].rearrange("p h q -> p (h q)"), sc[:],
                            Exp, bias=bias_sb[:], scale=float(SCALE))
                    else:
                        ex = ex_pool.tile([128, 1024], bf16, tag="ex", name="ex")
                        if band.start == 0:
                            nc.scalar.activation(ex[:], sc[:], Exp, bias=bias_sb[:], scale=float(SCALE))
                        else:
                            sc3 = sc[:].rearrange("p (h q) -> p h q", h=2)[:, :, band.start:band.stop]
                            ex3 = ex[:].rearrange("p (h q) -> p h q", h=2)[:, :, band.start:band.stop]
                            nc.scalar.activation(ex3, sc3, Exp, bias=bias_sb[:], scale=float(SCALE))
                        if d >= 0:
                            mband = slice(128 * d, 128 * (d + 1))
                            exm = ex[:].rearrange("p (h q) -> p h q", h=2)[:, :, mband]
                            mk3 = mk_sb[:].rearrange("p (h q) -> p h q", h=1).broadcast_to((128, 2, 128))
                            nc.vector.tensor_tensor(exm, exm, mk3, MUL)
                    ready = None
                    if j < ndiag:
                        if j % 2 == 1:
                            ready = ("dr", j // 2, ex2)
                    else:
                        ready = ("diag", j, band, ex)
                    if ready is not None:
                        pendq.append(ready)
                        if len(pendq) > 1:
                            bl = state["blocks_left"]
                            if bl > 1:
                                emit_budget(sum(it[3] for it in filler) / bl)
                            emit_ctx(pendq.pop(0))
                    state["blocks_left"] -= 1
                    if j == nblk - 1 and nxt is not None:
                        # next chunk's projections run in this chunk's
                        # boundary lull (pend flush + normalize latency)
                        force_sel("q", nxt[0], nxt[1])
                        force_sel("k", nxt[0], nxt[1])
                        force_sel("v", nxt[0], nxt[1])
                for p in pendq:
                    emit_ctx(p)
                # normalize: evacuate the ctx psum to SBUF immediately (frees the
                # pcx bank for the next chunk), then run the recip/broadcast/mult
                # chain out of SBUF off the PE's critical path.
                # c=0 -> bf16 CTX (unscaled); c>=1 -> fp8 CTX2 (x8)
                craw = {}
                bcs = {}
                for h in range(2):
                    craw[h] = nrm_pool.tile([65, 512], f32, tag="craw", name="craw")
                    nc.vector.tensor_copy(craw[h][:], ps_ctx[h][0:65, :])
                for h in range(2):
                    den = dv_pool.tile([1, 512], f32, tag="den")
                    if c == 0:
                        nc.vector.tensor_copy(den[:], craw[h][64:65, :])
                    else:
                        nc.vector.tensor_scalar_mul(den[:], craw[h][64:65, :], 0.125)
                    rec = dv_pool.tile([1, 512], f32, tag="rec")
                    nc.vector.reciprocal_approx_fast(out=rec[:], in_=den[:])
                    bcs[h] = nrm_pool.tile([64, 512], f32, tag="bc", name="bc")
                    nc.gpsimd.partition_broadcast(bcs[h][:], rec[:])
                # the multiplies are deferred into the filler stream so the DVE
                # queue never blocks waiting on the pool broadcast at a chunk
                # boundary (the next chunk's mask-mults would convoy behind it)
                def norm_mult(pair=pair, c=c, craw=craw, bcs=bcs):
                    for h in range(2):
                        rows = slice(64 * h, 64 * (h + 1))
                        if c == 0:
                            nc.vector.tensor_tensor(CTX[rows, pair, :], craw[h][0:64, :], bcs[h][:], MUL)
                        else:
                            nc.vector.tensor_tensor(
                                CTX2[rows, pair, 512 * c:512 * (c + 1)],
                                craw[h][0:64, :], bcs[h][:], MUL)
                filler.append(["n", pair, c, 0.1, norm_mult])
                # both pairs' chunk c done -> output projection quanta
                if chunks_done[c] == 2:
                    obholder = {}
                    last = (c == CHUNK_ORDER[-1][1])
                    for tb in range(4 * c, 4 * (c + 1)):
                        for e in range(2):
                            def ph3_quant(tb=tb, e=e, obholder=obholder, last=last):
                                force_sel("n", 0, 3)
                                force_sel("n", 1, 3)
                                grp, idx = divmod(tb, 4)
                                if idx == 0 and e == 0:
                                    obholder["ob"] = ob_pool.tile([128, 4, 1024], bf16, tag="ob", name="ob")
                                ob = obholder["ob"]
                                po = p1.tile([128, 512], f32, tag="p1", name="po")
                                if tb < 4:
                                    for pr in range(2):
                                        nc.tensor.matmul(
                                            po[:],
                                            CTX[:, pr, 128 * tb:128 * (tb + 1)],
                                            wo_sb[:, pr, 512 * e:512 * (e + 1)],
                                            start=(pr == 0), stop=(pr == 1),
                                        )
                                    if last:
                                        nc.scalar.activation(ob[:, idx, 512 * e:512 * (e + 1)], po[:], CopyFn)
                                    else:
                                        nc.vector.tensor_copy(ob[:, idx, 512 * e:512 * (e + 1)], po[:])
                                else:
                                    nc.tensor.matmul(
                                        po[:],
                                        CTX2[:, :, 128 * tb:128 * (tb + 1)],
                                        wo8_sb[:, :, 512 * e:512 * (e + 1)],
                                        start=True, stop=True,
                                        perf_mode=DR,
                                    )
                                    if last:
                                        nc.scalar.activation(
                                            ob[:, idx, 512 * e:512 * (e + 1)], po[:], CopyFn,
                                            scale=1.0 / 128.0)
                                    else:
                                        nc.vector.tensor_scalar_mul(
                                            ob[:, idx, 512 * e:512 * (e + 1)], po[:], 1.0 / 128.0)
                                if e == 1:
                                    nc.gpsimd.dma_start(
                                        out=outp[128 * grp:128 * (grp + 1), 1024 * idx:1024 * (idx + 1)],
                                        in_=ob[:, idx])
                            filler.append(["o", -1, 99, 0.45, ph3_quant])

            # ---- schedule
            CHUNK_ORDER = [(0, 0), (1, 0), (0, 1), (1, 1), (0, 3), (1, 3), (0, 2), (1, 2)]
            chunks_done = {c: 0 for c in range(4)}
            for tcn in range(4):
                for pair in range(2):
                    quanta = make_quanta(tcn, pair)
                    if tcn == 0 and pair == 0:
                        for it in quanta:
                            if it[0] in ("q", "k"):
                                it[4]()
                            else:
                                filler.append(it)
                    else:
                        filler.extend(quanta)

            for i, (pair, c) in enumerate(CHUNK_ORDER):
                chunks_done[c] += 1
                nxt = CHUNK_ORDER[i + 1] if i + 1 < len(CHUNK_ORDER) else None
                ph2_chunk(pair, c, nxt)
            while filler:
                filler.pop(0)[4]()

    nc.compile()
    return nc


def _host_inputs(q, k, v, mask, w_q, w_k, w_v, w_o):
    import ml_dtypes

    nbf = ml_dtypes.bfloat16
    nf8 = getattr(ml_dtypes, "float8_e4m3fn", None) or ml_dtypes.float8_e4m3

    def arrange_x(x):
        # [T, D] tokens-major -> [tcn, partition(d_low), featblock(a)*token]
        x2 = np.asarray(x, dtype=np.float32).reshape(T, D).T.astype(nbf)  # [D, T]
        x4 = x2.reshape(8, 128, 4, 512)             # (a, p, tcn, t)
        return np.ascontiguousarray(x4.transpose(2, 1, 0, 3).reshape(512, 4096))

    def arrange_w(wT):
        # wT [D, DPC] -> [128, pair*8*128] partition-major image of the SBUF tile
        w4 = wT.astype(nbf).reshape(8, 128, 2, 128)  # (a, p, pair, d)
        return np.ascontiguousarray(w4.transpose(1, 2, 0, 3).reshape(128, 2048))

    w_q = np.asarray(w_q, dtype=np.float32)
    w_k = np.asarray(w_k, dtype=np.float32)
    w_v = np.asarray(w_v, dtype=np.float32)
    w_o = np.asarray(w_o, dtype=np.float32)
    mask2d = np.asarray(mask).reshape(S, S)

    # single 128x128 tril mask for the mixed band of every diagonal block:
    # valid(r, u) = mask2d[u, r] on the leading 128x128 (= u >= r for causal)
    mk = np.ascontiguousarray(mask2d[0:128, 0:128].T.astype(nbf))

    xarr = {}
    for b in range(B):
        xarr[b] = (
            arrange_x(np.asarray(q)[b]),
            arrange_x(np.asarray(k)[b]),
            arrange_x(np.asarray(v)[b]),
        )

    in_maps = []
    for m in range(N_CORES):
        b, g = divmod(m, 4)
        sl = slice(DPC * g, DPC * (g + 1))
        q4, k4, v4 = xarr[b]
        # w_o[:, sl].T is [256, 1024]; split into two 128-row pair slabs
        wo2 = w_o[:, sl].T.reshape(2, 128, 1024).transpose(1, 0, 2)
        in_maps.append({
            "qT": q4,
            "kT": k4,
            "vT": v4,
            "wqT": arrange_w(w_q[sl, :].T),
            "wkT": arrange_w(w_k[sl, :].T),
            "wvT": arrange_w(w_v[sl, :].T),
            "woT": np.ascontiguousarray(wo2.astype(nbf).reshape(128, 2048)),
            "wo8T": np.ascontiguousarray((wo2 * 16.0).astype(nf8).reshape(128, 2048)),
            "msk": mk,
        })
    return in_maps


def kernel(q, k, v, mask, w_q, w_k, w_v, w_o, _trace=False, _results=None):
    in_maps = _host_inputs(q, k, v, mask, w_q, w_k, w_v, w_o)
    if "nc" not in _CACHED:
        _CACHED["nc"] = build_nc()
    nc = _CACHED["nc"]
    res = run_bass_kernel_spmd(
        nc, in_maps, core_ids=list(range(N_CORES)), trace=_trace
    )
    if _results is not None:
        _results.append(res)
    out = np.zeros((B, 512, 4096), dtype=np.float32)
    for m in range(N_CORES):
        out[m // 4] += np.asarray(res.results[m]["outp"], dtype=np.float32)
    # [group, partition, tokblock*feat] -> [B, S, D]
    out = out.reshape(B, 4, 128, 4, 1024).transpose(0, 1, 3, 2, 4).reshape(B, S, D)
    return out
